# revision 51
# baseline (speedup 1.0000x reference)
"""BiologicalAttention Trainium2 kernel (v3).

Sharding: head-parallel across 8 cores. Core c computes head h=c for both
batches (b=0,1). Each core produces a partial output contribution
ctx_h @ Wo[h_slice, :] of shape [B*S, HIDDEN] in fp16; the host sums the
8 partials in float64 and adds bo.

Per-core pipeline per batch (S=2048, Dh=128; matmuls fp16-in/f32-acc):
  1. qT/kT/vT = W^T @ xT on PE (1.25/sqrt(Dh) folded into qT); vblk
     (v in [j,d] layout) via DMA-engine transpose of vT. phA(0) is
     emitted between proj(0) and proj(1) so threshold work overlaps the
     second batch's projection (batch-1 scores pool is created after
     the xt pool frees its SBUF).
  2. phA per batch, pipelined per-tile in two 8-tile halves:
     S = q @ k^T (PE) -> PSUM evict with fused row sums -> mu (ACT,
     with a few tiles on DVE); statistical top-k threshold
     (k=409 = 20% of 2048):
       sigma_i ~= ||q_i|| * sqrt(sum k^2/(S*Dh)) (one DVE tt per half +
       tiny PE column-sum matmuls; k Frobenius norm via stt accum)
       t0 = mu + z0*sigma, z0 = Phi^-1(0.8) = 0.8416
     one count pass at t0 over half the key columns (DVE ts is_ge +
     accum) and a Newton step off the Gaussian density:
       t1 = t0 + (2*cnt - 409)*sigma/(S*phi(z0))
     mask m24 = (S >= t1)*0.24 (DVE ts), s2 = (1+m24)*S in place
     (split: DVE stt / two Pool tensor_tensor — Pool stt/ts fail the
     HW ISA engine check).
  3. pooled = colmean(s2) via ones-vector matmul on PE; li = width-3
     conv row ops; broadcast li to 128 partitions via PE outer product.
  4. av per 512-query chunk: s3 = s2*li128 in place (DVE/Pool split);
     exp(s3) in place on ACT with fused row-sum accum -> Z (no zsum
     matmuls); P^T blocks via DMA-engine transpose (SBUF->SBUF, 112ns
     per [128,128] block on the 4 SWDGE queues) -> AV matmul
     ctxT = v^T @ P^T on PE (P unnormalized; values stay in fp16
     range) -> outproj ctxT^T @ Wo_h on PE, with 1/Z applied per query
     row in the PSUM eviction (DVE ts with per-partition ptr) ->
     [128,1024] fp16 DMA out.

Emission staggers the batches so batch-1 threshold work (DVE/Pool)
overlaps batch-0 attention*V (ACT/PE/DMA) and vice versa. Engine
choices come from the CoreSim cost model: DVE ts 694ns / tt 1227 /
stt 2294, Pool tt 1807 flat, ACT exp+accum 2179 per [128,2048] tile.
"""

import sys
from contextlib import ExitStack

import numpy as np

B, S, HIDDEN = 2, 2048, 1024
HEADS, DH = 8, 128
P = 128
NT = S // P            # 16 i-tiles per batch
NJC = S // 512         # 4 chunks of 512
NEC = HIDDEN // P      # 8 contraction tiles for projections
SCALE = float(1.25 / np.sqrt(DH))
TOPK = 409
Z0 = 0.8416            # Phi^-1(1 - 409/2048)
PHI0 = float(np.exp(-0.5 * Z0 * Z0) / np.sqrt(2 * np.pi))
C_NEWT = float(1.0 / (S * PHI0))   # Newton step: dt = (cnt-TOPK)*sigma*C_NEWT
N_NEWTON = 1                       # 0 = statistical threshold only


def _bass_modules():
    sys.path.insert(0, "/opt/trn_rl_repo")
    import concourse.bacc as bacc
    import concourse.mybir as mybir
    import concourse.tile as tile
    from concourse import masks
    from concourse.bass_utils import run_bass_kernel_spmd

    return bacc, mybir, tile, masks, run_bass_kernel_spmd


def build(nc, tile, mybir, masks):
    AF = mybir.ActivationFunctionType
    OP = mybir.AluOpType
    f32 = mybir.dt.float32
    f16 = mybir.dt.float16

    xt_d = nc.dram_tensor("xt", [HIDDEN, B * S], f16, kind="ExternalInput").ap()
    wq_d = nc.dram_tensor("wq", [HIDDEN, DH], f16, kind="ExternalInput").ap()
    wk_d = nc.dram_tensor("wk", [HIDDEN, DH], f16, kind="ExternalInput").ap()
    wv_d = nc.dram_tensor("wv", [HIDDEN, DH], f16, kind="ExternalInput").ap()
    wo_d = nc.dram_tensor("wo", [DH, HIDDEN], f16, kind="ExternalInput").ap()
    bq_d = nc.dram_tensor("bq", [DH, 1], f32, kind="ExternalInput").ap()
    bk_d = nc.dram_tensor("bk", [DH, 1], f32, kind="ExternalInput").ap()
    bv_d = nc.dram_tensor("bv", [DH, 1], f32, kind="ExternalInput").ap()
    cw_d = nc.dram_tensor("cw", [1, 3], f32, kind="ExternalInput").ap()
    cb_d = nc.dram_tensor("cb", [1, 1], f32, kind="ExternalInput").ap()
    out_d = nc.dram_tensor("out", [B * S, HIDDEN], f16, kind="ExternalOutput").ap()

    with tile.TileContext(nc) as tc, ExitStack() as es:
        const = es.enter_context(tc.tile_pool(name="const", bufs=1))
        ones = const.tile([P, 1], f16, name="ones")
        nc.gpsimd.memset(ones[:], 1.0)
        onesr = const.tile([1, P], f16, name="onesr")
        nc.gpsimd.memset(onesr[:], 1.0)
        ones32 = const.tile([P, 1], f32, name="ones32")
        nc.gpsimd.memset(ones32[:], 1.0)
        onesr32 = const.tile([1, P], f32, name="onesr32")
        nc.gpsimd.memset(onesr32[:], 1.0)
        wq = const.tile([P, NEC * DH], f16, name="wq")
        wk = const.tile([P, NEC * DH], f16, name="wk")
        wv = const.tile([P, NEC * DH], f16, name="wv")
        wo = const.tile([P, HIDDEN], f16, name="wo")
        for et in range(NEC):
            nc.sync.dma_start(wq[:, et * DH:(et + 1) * DH], wq_d[et * P:(et + 1) * P, :])
        bq = const.tile([P, 1], f32, name="bq")
        bk = const.tile([P, 1], f32, name="bk")
        bv = const.tile([P, 1], f32, name="bv")
        nc.sync.dma_start(bq[:], bq_d[:, :])
        nc.sync.dma_start(bk[:], bk_d[:, :])
        nc.sync.dma_start(bv[:], bv_d[:, :])
        cw = const.tile([1, 3], f32, name="cw")
        cb = const.tile([1, 1], f32, name="cb")
        nc.sync.dma_start(cw[:], cw_d[:, :])
        nc.sync.dma_start(cb[:], cb_d[:, :])

        # --- psum pools: 8 banks total ---
        ps_s = es.enter_context(tc.tile_pool(name="ps_s", bufs=2, space="PSUM"))
        ps_av = es.enter_context(tc.tile_pool(name="ps_av", bufs=2, space="PSUM"))
        ps_z = es.enter_context(tc.tile_pool(name="ps_z", bufs=1, space="PSUM"))
        ps_sm = es.enter_context(tc.tile_pool(name="ps_sm", bufs=1, space="PSUM"))

        qkv = es.enter_context(tc.tile_pool(name="qkv", bufs=1))
        qT = [qkv.tile([P, S], f16, tag=f"qT{b}", name=f"qT{b}") for b in range(B)]
        kT = [qkv.tile([P, S], f16, tag=f"kT{b}", name=f"kT{b}") for b in range(B)]
        vblk = [qkv.tile([P, S], f16, tag=f"vblk{b}", name=f"vblk{b}") for b in range(B)]

        # ---- attention state (created before xt so phA(0) can be
        # emitted between the two batches' projections) ----
        spool = {0: es.enter_context(tc.tile_pool(name="scores0", bufs=NT))}
        small = es.enter_context(tc.tile_pool(name="small", bufs=1))
        mpool = es.enter_context(tc.tile_pool(name="mask", bufs=2))
        pts_pool = es.enter_context(tc.tile_pool(name="pts", bufs=8))
        outp = es.enter_context(tc.tile_pool(name="outp", bufs=3))

        STAT = ["musum", "mu", "sig", "t0", "t1", "cnt", "tmp1", "zsum", "zrec"]
        st = {b: {nm: small.tile([P, NT], f32, tag=f"{nm}{b}", name=f"{nm}{b}")
                  for nm in STAT} for b in range(B)}
        for b in range(B):
            st[b]["musum4"] = small.tile(
                [P, 2 * NT], f32, tag=f"musum4{b}", name=f"musum4{b}")
            st[b]["w0"] = small.tile([P, 1], f32, tag=f"w0{b}", name=f"w0{b}")
            st[b]["s1"] = small.tile([1, 1], f32, tag=f"s1{b}", name=f"s1{b}")
        li128 = {b: small.tile([P, S], f16, tag=f"li128{b}", name=f"li128{b}")
                 for b in range(B)}
        qscr_sh = small.tile([P, 1024], f16, tag="qscr", name="qscr")
        qscr = {0: qscr_sh, 1: qscr_sh}
        ctx_pool = es.enter_context(tc.tile_pool(name="ctx", bufs=2))
        Sti = {}

        # ---- phase A: S = q @ k^T, eviction (+mu accum), count, Newton,
        # mask, emphasis — pipelined per tile in two 8-tile halves so the
        # count/mask work on early tiles overlaps matmuls of later ones.
        def phA(b):
            Sti[b] = [spool[b].tile([P, S], f16, tag="score", name=f"sc{b}_{i}")
                      for i in range(NT)]
            v = st[b]
            musum4 = v["musum4"]
            for half in range(8):
                its = range(half * 2, half * 2 + 2)
                hs = slice(half * 2, half * 2 + 2)
                for it in its:
                    for jc2 in range(NJC // 2):
                        ps = ps_s.tile([P, 1024], f32, tag="ps_s", name="ps")
                        for h2 in range(2):
                            jc = jc2 * 2 + h2
                            nc.tensor.matmul(
                                ps[:, h2 * 512:(h2 + 1) * 512],
                                qT[b][:, it * P:(it + 1) * P],
                                kT[b][:, jc * 512:(jc + 1) * 512],
                                start=True, stop=True,
                            )
                        acc = musum4[:, jc2 * NT + it: jc2 * NT + it + 1]
                        dst = Sti[b][it][:, jc2 * 1024:(jc2 + 1) * 1024]
                        on_dve = (it % 4 == 0) if b == 0 else (it in (5, 11))
                        if on_dve:
                            nc.vector.tensor_scalar(
                                dst, ps[:], 1.0, None, OP.mult, OP.add,
                                accum_out=acc)
                        else:
                            nc.scalar.activation(dst, ps[:], AF.Copy,
                                                 accum_out=acc)
                # t0 = mu + sig*w0 for this half's 8 columns
                nc.vector.tensor_add(v["mu"][:, hs], musum4[:, hs],
                                     musum4[:, NT + half * 2: NT + half * 2 + 2])
                nc.vector.tensor_scalar(v["mu"][:, hs], v["mu"][:, hs],
                                        1.0 / S, None, OP.mult)
                nc.vector.tensor_scalar(v["sig"][:, hs], v["sig"][:, hs],
                                        v["w0"][:, 0:1], None, OP.mult)
                nc.vector.tensor_add(v["t0"][:, hs], v["mu"][:, hs],
                                     v["sig"][:, hs])
                if N_NEWTON:
                    # count pass at t0 on the first half of the key columns
                    # (order statistics of an iid half-sample)
                    for it in its:
                        mdump = mpool.tile([P, S], f16, tag="m24", name="mdump")
                        nc.vector.tensor_scalar(
                            mdump[:, 0:512], Sti[b][it][:, 0:512],
                            v["t0"][:, it:it + 1], None,
                            OP.is_ge, OP.add, accum_out=v["cnt"][:, it:it + 1],
                        )
                    # Newton: t1 = t0 + (2*cnt-TOPK)*C_NEWT/Z0 * sig
                    nc.vector.tensor_scalar(
                        v["tmp1"][:, hs], v["cnt"][:, hs], float(TOPK / 4),
                        float(4.0 * C_NEWT / Z0), OP.subtract, OP.mult)
                    nc.vector.tensor_tensor(v["tmp1"][:, hs], v["tmp1"][:, hs],
                                            v["sig"][:, hs], OP.mult)
                    nc.vector.tensor_add(v["t1"][:, hs], v["t0"][:, hs],
                                         v["tmp1"][:, hs])
                else:
                    nc.vector.tensor_copy(v["t1"][:, hs], v["t0"][:, hs])
                # mask m24 = (S>=t1)*0.24 (DVE ts), then s2 = (1+m24)*S,
                # split DVE stt / Pool 2x tensor_tensor (Pool stt/ts fail
                # the HW ISA engine check).
                for it in its:
                    m24 = mpool.tile([P, S], f16, tag="m24", name="m24")
                    nc.vector.tensor_scalar(
                        m24[:], Sti[b][it][:], v["t1"][:, it:it + 1], 0.24,
                        OP.is_ge, OP.mult)
                    s2_dve = (it % 4 == 0) if b == 0 else (it % 3 == 0)
                    if s2_dve:
                        nc.vector.scalar_tensor_tensor(
                            Sti[b][it][:], m24[:], 1.0, Sti[b][it][:],
                            OP.add, OP.mult)
                    else:
                        nc.gpsimd.tensor_tensor(
                            m24[:], Sti[b][it][:], m24[:], OP.mult)
                        nc.gpsimd.tensor_tensor(
                            Sti[b][it][:], Sti[b][it][:], m24[:], OP.add)

        # ---- sigma_i ~ ||q_i|| * sqrt(sum k^2/(S*Dh)); t0 = mu + z0*sig ----
        def qstats(b):
            v = st[b]
            # q2 = qT*qT elementwise (f16); column sums via tiny PE matmuls
            q2s = ps_sm.tile([P, NT], f32, tag="ps_sm", name="q2s")
            for h2 in range(2):
                nc.vector.tensor_tensor(
                    qscr[b][:], qT[b][:, h2 * 1024:(h2 + 1) * 1024],
                    qT[b][:, h2 * 1024:(h2 + 1) * 1024], OP.mult)
                for it8 in range(8):
                    it = h2 * 8 + it8
                    nc.tensor.matmul(
                        q2s[:, it:it + 1],
                        qscr[b][:, it8 * P:(it8 + 1) * P], ones[:],
                        start=True, stop=True,
                    )
            # sig <- sqrt(||q_i||^2) (w0 factor applied below)
            nc.scalar.activation(v["sig"][:], q2s[:], AF.Sqrt)
            # k Frobenius norm: accum (kT*kT) rows -> [P,1]x2 halves, reduce
            for h2 in range(2):
                nc.vector.scalar_tensor_tensor(
                    qscr[b][:], kT[b][:, h2 * 1024:(h2 + 1) * 1024], 1.0,
                    kT[b][:, h2 * 1024:(h2 + 1) * 1024], OP.mult, OP.mult,
                    accum_out=v["tmp1"][:, h2:h2 + 1],
                )
            nc.vector.tensor_add(v["tmp1"][:, 0:1], v["tmp1"][:, 0:1],
                                 v["tmp1"][:, 1:2])
            kks = ps_z.tile([1, 16], f32, tag="ps_z", name="kks")
            nc.tensor.matmul(kks[0:1, 0:1], v["tmp1"][:, 0:1], ones32[:],
                             start=True, stop=True)
            nc.vector.tensor_copy(v["s1"][:], kks[0:1, 0:1])
            w0p = ps_sm.tile([P, NT], f32, tag="ps_sm", name="w0p")
            nc.tensor.matmul(w0p[:, 0:1], onesr32[:], v["s1"][:],
                             start=True, stop=True)
            # w0 = z0 * sqrt(kks/(S*Dh)) = sqrt(z0^2/(S*Dh) * kks)
            nc.scalar.activation(v["w0"][:], w0p[:, 0:1], AF.Sqrt,
                                 scale=float(Z0 * Z0 / (S * DH)))

        def pooled_li(b):
            if b == 0:
                nc.sync.dma_start(wo[:], wo_d[:, :])
            pooled = small.tile([1, S + 2], f16, tag="rowA", name="pooled")
            li = small.tile([1, S], f16, tag="rowB", name="li")
            nc.gpsimd.memset(pooled[0:1, 0:1], 0.0)
            nc.gpsimd.memset(pooled[0:1, S + 1:S + 2], 0.0)
            for jc in range(NJC):
                ps = ps_z.tile([1, 512], f32, tag="ps_z", name="psp")
                for it in range(NT):
                    nc.tensor.matmul(
                        ps[:], ones[:],
                        Sti[b][it][:, jc * 512:(jc + 1) * 512],
                        start=(it == 0), stop=(it == NT - 1),
                    )
                nc.scalar.activation(
                    pooled[0:1, 1 + jc * 512:1 + (jc + 1) * 512], ps[:],
                    AF.Copy, scale=1.0 / S,
                )
            nc.vector.tensor_scalar(
                li[:], pooled[0:1, 1:S + 1], cw[0:1, 1:2], cb[0:1, 0:1],
                OP.mult, OP.add)
            nc.vector.scalar_tensor_tensor(
                li[:], pooled[0:1, 0:S], cw[0:1, 0:1], li[:], OP.mult, OP.add)
            nc.vector.scalar_tensor_tensor(
                li[:], pooled[0:1, 2:S + 2], cw[0:1, 2:3], li[:], OP.mult, OP.add)
            for jc in range(NJC):
                psb = ps_s.tile([P, 512], f32, tag="ps_s", name="psb")
                nc.tensor.matmul(
                    psb[:], onesr[:], li[0:1, jc * 512:(jc + 1) * 512],
                    start=True, stop=True,
                )
                nc.vector.tensor_copy(li128[b][:, jc * 512:(jc + 1) * 512], psb[:])

        def s3ic(b, ic):
            # s3 = s2 * li128 in place, split DVE/Pool to balance load
            for ib in range(4):
                it = ic * 4 + ib
                if it % 2 == 0:
                    nc.vector.tensor_tensor(
                        Sti[b][it][:], Sti[b][it][:], li128[b][:], OP.mult)
                else:
                    nc.gpsimd.tensor_tensor(
                        Sti[b][it][:], Sti[b][it][:], li128[b][:], OP.mult)

        # ---- exp + Z (fused accum) + AV via DMA transpose + outproj ----
        # P^T is consumed unnormalized (ctxT scaled by 2^-12 to stay in
        # fp16 range); 1/Z is applied per query row in the outproj PSUM
        # eviction, whose output rows are queries (DVE ts with ptr).
        CTX_SC = 1.0
        def av_ic(b, ic):
                v = st[b]
                s3ic(b, ic)
                for ib in range(4):
                    it = ic * 4 + ib
                    nc.scalar.activation(
                        Sti[b][it][:], Sti[b][it][:], AF.Exp,
                        accum_out=v["zsum"][:, it:it + 1])
                c4 = slice(ic * 4, ic * 4 + 4)
                with nc.allow_low_precision(reason="1/Z f32 tiny tile"):
                    nc.vector.reciprocal(v["zrec"][:, c4], v["zsum"][:, c4])
                pav = ps_av.tile([P, 512], f32, tag="ps_av", name="pav")
                for jt in range(NT):
                    pts = pts_pool.tile([P, 512], f16, tag="pts", name="pts")
                    for ib in range(4):
                        it = ic * 4 + ib
                        nc.sync.dma_start_transpose(
                            pts[:, ib * P:(ib + 1) * P],
                            Sti[b][it][:, jt * P:(jt + 1) * P])
                    nc.tensor.matmul(
                        pav[:], vblk[b][:, jt * P:(jt + 1) * P], pts[:],
                        start=(jt == 0), stop=(jt == NT - 1),
                    )
                ctxc = ctx_pool.tile([P, 512], f16, tag="ctx", name="ctxc")
                nc.scalar.activation(ctxc[:], pav[:], AF.Copy)
                for ib in range(4):
                    ibg = ic * 4 + ib
                    po = ps_s.tile([P, 1024], f32, tag="ps_s", name="po")
                    for h2 in range(2):
                        nc.tensor.matmul(
                            po[:, h2 * 512:(h2 + 1) * 512],
                            ctxc[:, ib * P:(ib + 1) * P],
                            wo[:, h2 * 512:(h2 + 1) * 512],
                            start=True, stop=True,
                        )
                    ot = outp.tile([P, 1024], f16, tag="out", name="ot")
                    if b == 1:
                        nc.scalar.activation(
                            ot[:], po[:], AF.Copy,
                            scale=v["zrec"][:, ibg:ibg + 1])
                    else:
                        nc.vector.tensor_scalar(
                            ot[:], po[:], v["zrec"][:, ibg:ibg + 1], None,
                            OP.mult)
                    nc.sync.dma_start(
                        out_d[b * S + ibg * P: b * S + (ibg + 1) * P, :], ot[:])

        # ---- phase 1+A interleaved: proj(0), qstats(0), phA(0) start,
        # proj(1); batch-1 scores pool is created only after the xt pool
        # frees its SBUF region.
        with tc.tile_pool(name="xt", bufs=12) as xt_pool:
            vT = [xt_pool.tile([P, S], f16, tag=f"vT{b}", name=f"vT{b}", bufs=1)
                  for b in range(B)]

            def proj(b):
                for jc in range(NJC // 2):
                    xts = []
                    for et in range(NEC):
                        t = xt_pool.tile([P, 1024], f16, tag="xts", name="xts")
                        nc.sync.dma_start(
                            t[:],
                            xt_d[et * P:(et + 1) * P,
                                 b * S + jc * 1024: b * S + (jc + 1) * 1024])
                        xts.append(t)
                    if b == 0 and jc == 0:
                        for et in range(NEC):
                            nc.sync.dma_start(wk[:, et * DH:(et + 1) * DH],
                                              wk_d[et * P:(et + 1) * P, :])
                            nc.sync.dma_start(wv[:, et * DH:(et + 1) * DH],
                                              wv_d[et * P:(et + 1) * P, :])
                    for dst, w, bias, scl in (
                            (qT[b], wq, bq, SCALE), (kT[b], wk, bk, 1.0),
                            (vT[b], wv, bv, 1.0)):
                        ps = ps_s.tile([P, 1024], f32, tag="ps_s", name="ps")
                        for h2 in range(2):
                            for et in range(NEC):
                                nc.tensor.matmul(
                                    ps[:, h2 * 512:(h2 + 1) * 512],
                                    w[:, et * DH:(et + 1) * DH],
                                    xts[et][:, h2 * 512:(h2 + 1) * 512],
                                    start=(et == 0), stop=(et == NEC - 1),
                                )
                        nc.scalar.activation(
                            dst[:, jc * 1024:(jc + 1) * 1024], ps[:],
                            AF.Identity, bias=bias[:, 0:1], scale=scl,
                        )
                for jt in range(NT):
                    nc.sync.dma_start_transpose(
                        vblk[b][:, jt * P:(jt + 1) * P],
                        vT[b][:, jt * P:(jt + 1) * P])

            proj(0)
            qstats(0)
            phA(0)
            proj(1)
            qstats(1)
        spool[1] = es.enter_context(tc.tile_pool(name="scores1", bufs=NT))
        phA(1)
        pooled_li(0)
        pooled_li(1)
        for ic in range(NJC):
            av_ic(0, ic)
        for ic in range(NJC):
            av_ic(1, ic)

    return nc


def prep_core_inputs(inputs, c):
    """Host-side slice of the full inputs for core c (head h=c)."""
    x = np.ascontiguousarray(inputs["x"], dtype=np.float32)
    sl = slice(c * DH, (c + 1) * DH)
    return {
        "xt": np.ascontiguousarray(x.reshape(B * S, HIDDEN).T.astype(np.float16)),
        "wq": np.ascontiguousarray(inputs["Wq"][:, sl], dtype=np.float16),
        "wk": np.ascontiguousarray(inputs["Wk"][:, sl], dtype=np.float16),
        "wv": np.ascontiguousarray(inputs["Wv"][:, sl], dtype=np.float16),
        "wo": np.ascontiguousarray(inputs["Wo"][sl, :], dtype=np.float16),
        "bq": np.ascontiguousarray(
            inputs["bq"][sl].reshape(DH, 1) * (1.25 / np.sqrt(DH)),
            dtype=np.float32),
        "bk": np.ascontiguousarray(inputs["bk"][sl].reshape(DH, 1), dtype=np.float32),
        "bv": np.ascontiguousarray(inputs["bv"][sl].reshape(DH, 1), dtype=np.float32),
        "cw": np.ascontiguousarray(inputs["conv_w"][c].reshape(1, 3), dtype=np.float32),
        "cb": np.ascontiguousarray(inputs["conv_b"][c].reshape(1, 1), dtype=np.float32),
    }


def build_nc():
    bacc, mybir, tile, masks, _ = _bass_modules()
    nc = bacc.Bacc("TRN2", target_bir_lowering=False, num_swdge_queues=4)
    build(nc, tile, mybir, masks)
    nc.compile()
    return nc


def kernel(**inputs):
    bacc, mybir, tile, masks, run_bass_kernel_spmd = _bass_modules()
    nc = build_nc()
    in_maps = [prep_core_inputs(inputs, c) for c in range(HEADS)]
    res = run_bass_kernel_spmd(nc, in_maps, core_ids=list(range(HEADS)))
    out = np.zeros((B * S, HIDDEN), dtype=np.float64)
    for c in range(HEADS):
        out += res.results[c]["out"].astype(np.float64)
    out = out + np.asarray(inputs["bo"], dtype=np.float64)[None, :]
    return out.reshape(B, S, HIDDEN).astype(np.float32)


if __name__ == "__main__":
    import reference as R

    inputs = {k: np.asarray(v) for k, v in R.setup_inputs().items()}
    got = kernel(**inputs)
    exp = np.asarray(R.reference(**inputs))
    d = np.abs(got - exp)
    print("absmax", d.max(), "rel", d.max() / np.abs(exp).max())


# revision 52
# speedup vs baseline: 1.0103x; 1.0103x over previous
"""BiologicalAttention Trainium2 kernel (v3).

Sharding: head-parallel across 8 cores. Core c computes head h=c for both
batches (b=0,1). Each core produces a partial output contribution
ctx_h @ Wo[h_slice, :] of shape [B*S, HIDDEN] in fp16; the host sums the
8 partials in float64 and adds bo.

Per-core pipeline per batch (S=2048, Dh=128; matmuls fp16-in/f32-acc):
  1. qT/kT/vT = W^T @ xT on PE (1.25/sqrt(Dh) folded into qT); vblk
     (v in [j,d] layout) via DMA-engine transpose of vT. phA(0) is
     emitted between proj(0) and proj(1) so threshold work overlaps the
     second batch's projection (batch-1 scores pool is created after
     the xt pool frees its SBUF).
  2. phA per batch, pipelined per-tile in two 8-tile halves:
     S = q @ k^T (PE) -> PSUM evict with fused row sums -> mu (ACT,
     with a few tiles on DVE); statistical top-k threshold
     (k=409 = 20% of 2048):
       sigma_i ~= ||q_i|| * sqrt(sum k^2/(S*Dh)) (one DVE tt per half +
       tiny PE column-sum matmuls; k Frobenius norm via stt accum)
       t0 = mu + z0*sigma, z0 = Phi^-1(0.8) = 0.8416
     one count pass at t0 over half the key columns (DVE ts is_ge +
     accum) and a Newton step off the Gaussian density:
       t1 = t0 + (2*cnt - 409)*sigma/(S*phi(z0))
     mask m24 = (S >= t1)*0.24 (DVE ts), s2 = (1+m24)*S in place
     (split: DVE stt / two Pool tensor_tensor — Pool stt/ts fail the
     HW ISA engine check).
  3. pooled = colmean(s2) via ones-vector matmul on PE; li = width-3
     conv row ops; broadcast li to 128 partitions via PE outer product.
  4. av per 512-query chunk: s3 = s2*li128 in place (DVE/Pool split);
     exp(s3) in place on ACT with fused row-sum accum -> Z (no zsum
     matmuls); P^T blocks via DMA-engine transpose (SBUF->SBUF, 112ns
     per [128,128] block on the 4 SWDGE queues) -> AV matmul
     ctxT = v^T @ P^T on PE (P unnormalized; values stay in fp16
     range) -> outproj ctxT^T @ Wo_h on PE, with 1/Z applied per query
     row in the PSUM eviction (DVE ts with per-partition ptr) ->
     [128,1024] fp16 DMA out.

Emission staggers the batches so batch-1 threshold work (DVE/Pool)
overlaps batch-0 attention*V (ACT/PE/DMA) and vice versa. Engine
choices come from the CoreSim cost model: DVE ts 694ns / tt 1227 /
stt 2294, Pool tt 1807 flat, ACT exp+accum 2179 per [128,2048] tile.
"""

import sys
from contextlib import ExitStack

import numpy as np

B, S, HIDDEN = 2, 2048, 1024
HEADS, DH = 8, 128
P = 128
NT = S // P            # 16 i-tiles per batch
NJC = S // 512         # 4 chunks of 512
NEC = HIDDEN // P      # 8 contraction tiles for projections
SCALE = float(1.25 / np.sqrt(DH))
TOPK = 409
Z0 = 0.8416            # Phi^-1(1 - 409/2048)
PHI0 = float(np.exp(-0.5 * Z0 * Z0) / np.sqrt(2 * np.pi))
C_NEWT = float(1.0 / (S * PHI0))   # Newton step: dt = (cnt-TOPK)*sigma*C_NEWT
N_NEWTON = 1                       # 0 = statistical threshold only


def _bass_modules():
    sys.path.insert(0, "/opt/trn_rl_repo")
    import concourse.bacc as bacc
    import concourse.mybir as mybir
    import concourse.tile as tile
    from concourse import masks
    from concourse.bass_utils import run_bass_kernel_spmd

    return bacc, mybir, tile, masks, run_bass_kernel_spmd


def build(nc, tile, mybir, masks):
    AF = mybir.ActivationFunctionType
    OP = mybir.AluOpType
    f32 = mybir.dt.float32
    f16 = mybir.dt.float16

    xt_d = nc.dram_tensor("xt", [HIDDEN, B * S], f16, kind="ExternalInput").ap()
    wq_d = nc.dram_tensor("wq", [HIDDEN, DH], f16, kind="ExternalInput").ap()
    wk_d = nc.dram_tensor("wk", [HIDDEN, DH], f16, kind="ExternalInput").ap()
    wv_d = nc.dram_tensor("wv", [HIDDEN, DH], f16, kind="ExternalInput").ap()
    wo_d = nc.dram_tensor("wo", [DH, HIDDEN], f16, kind="ExternalInput").ap()
    bq_d = nc.dram_tensor("bq", [DH, 1], f32, kind="ExternalInput").ap()
    bk_d = nc.dram_tensor("bk", [DH, 1], f32, kind="ExternalInput").ap()
    bv_d = nc.dram_tensor("bv", [DH, 1], f32, kind="ExternalInput").ap()
    cw_d = nc.dram_tensor("cw", [1, 3], f32, kind="ExternalInput").ap()
    cb_d = nc.dram_tensor("cb", [1, 1], f32, kind="ExternalInput").ap()
    out_d = nc.dram_tensor("out", [B * S, HIDDEN], f16, kind="ExternalOutput").ap()

    with tile.TileContext(nc) as tc, ExitStack() as es:
        const = es.enter_context(tc.tile_pool(name="const", bufs=1))
        ones = const.tile([P, 1], f16, name="ones")
        nc.gpsimd.memset(ones[:], 1.0)
        onesr = const.tile([1, P], f16, name="onesr")
        nc.gpsimd.memset(onesr[:], 1.0)
        ones32 = const.tile([P, 1], f32, name="ones32")
        nc.gpsimd.memset(ones32[:], 1.0)
        onesr32 = const.tile([1, P], f32, name="onesr32")
        nc.gpsimd.memset(onesr32[:], 1.0)
        wq = const.tile([P, NEC * DH], f16, name="wq")
        wk = const.tile([P, NEC * DH], f16, name="wk")
        wv = const.tile([P, NEC * DH], f16, name="wv")
        wo = const.tile([P, HIDDEN], f16, name="wo")
        for et in range(NEC):
            nc.sync.dma_start(wq[:, et * DH:(et + 1) * DH], wq_d[et * P:(et + 1) * P, :])
        bq = const.tile([P, 1], f32, name="bq")
        bk = const.tile([P, 1], f32, name="bk")
        bv = const.tile([P, 1], f32, name="bv")
        nc.sync.dma_start(bq[:], bq_d[:, :])
        nc.sync.dma_start(bk[:], bk_d[:, :])
        nc.sync.dma_start(bv[:], bv_d[:, :])
        cw = const.tile([1, 3], f32, name="cw")
        cb = const.tile([1, 1], f32, name="cb")
        nc.sync.dma_start(cw[:], cw_d[:, :])
        nc.sync.dma_start(cb[:], cb_d[:, :])

        # --- psum pools: 8 banks total ---
        ps_s = es.enter_context(tc.tile_pool(name="ps_s", bufs=2, space="PSUM"))
        ps_av = es.enter_context(tc.tile_pool(name="ps_av", bufs=2, space="PSUM"))
        ps_z = es.enter_context(tc.tile_pool(name="ps_z", bufs=1, space="PSUM"))
        ps_sm = es.enter_context(tc.tile_pool(name="ps_sm", bufs=1, space="PSUM"))

        qkv = es.enter_context(tc.tile_pool(name="qkv", bufs=1))
        qT = [qkv.tile([P, S], f16, tag=f"qT{b}", name=f"qT{b}") for b in range(B)]
        kT = [qkv.tile([P, S], f16, tag=f"kT{b}", name=f"kT{b}") for b in range(B)]
        vblk = [qkv.tile([P, S], f16, tag=f"vblk{b}", name=f"vblk{b}") for b in range(B)]

        # ---- attention state (created before xt so phA(0) can be
        # emitted between the two batches' projections) ----
        spool = {0: es.enter_context(tc.tile_pool(name="scores0", bufs=NT))}
        small = es.enter_context(tc.tile_pool(name="small", bufs=1))
        mpool = es.enter_context(tc.tile_pool(name="mask", bufs=2))
        pts_pool = es.enter_context(tc.tile_pool(name="pts", bufs=8))
        outp = es.enter_context(tc.tile_pool(name="outp", bufs=3))

        STAT = ["musum", "mu", "sig", "t0", "t1", "cnt", "tmp1", "zsum", "zrec"]
        st = {b: {nm: small.tile([P, NT], f32, tag=f"{nm}{b}", name=f"{nm}{b}")
                  for nm in STAT} for b in range(B)}
        for b in range(B):
            st[b]["musum4"] = small.tile(
                [P, 2 * NT], f32, tag=f"musum4{b}", name=f"musum4{b}")
            st[b]["w0"] = small.tile([P, 1], f32, tag=f"w0{b}", name=f"w0{b}")
            st[b]["s1"] = small.tile([1, 1], f32, tag=f"s1{b}", name=f"s1{b}")
        li128 = {b: small.tile([P, S], f16, tag=f"li128{b}", name=f"li128{b}")
                 for b in range(B)}
        qscr_sh = small.tile([P, 1024], f16, tag="qscr", name="qscr")
        qscr = {0: qscr_sh, 1: qscr_sh}
        ctx_pool = es.enter_context(tc.tile_pool(name="ctx", bufs=2))
        Sti = {}

        # ---- phase A: S = q @ k^T, eviction (+mu accum), count, Newton,
        # mask, emphasis — pipelined per tile in two 8-tile halves so the
        # count/mask work on early tiles overlaps matmuls of later ones.
        def phA(b):
            Sti[b] = [spool[b].tile([P, S], f16, tag="score", name=f"sc{b}_{i}")
                      for i in range(NT)]
            v = st[b]
            musum4 = v["musum4"]
            for half in range(8):
                its = range(half * 2, half * 2 + 2)
                hs = slice(half * 2, half * 2 + 2)
                for it in its:
                    for jc2 in range(NJC // 2):
                        ps = ps_s.tile([P, 1024], f32, tag="ps_s", name="ps")
                        for h2 in range(2):
                            jc = jc2 * 2 + h2
                            nc.tensor.matmul(
                                ps[:, h2 * 512:(h2 + 1) * 512],
                                qT[b][:, it * P:(it + 1) * P],
                                kT[b][:, jc * 512:(jc + 1) * 512],
                                start=True, stop=True,
                            )
                        acc = musum4[:, jc2 * NT + it: jc2 * NT + it + 1]
                        dst = Sti[b][it][:, jc2 * 1024:(jc2 + 1) * 1024]
                        on_dve = (it % 4 == 0) if b == 0 else (it in (5, 11))
                        if on_dve:
                            nc.vector.tensor_scalar(
                                dst, ps[:], 1.0, None, OP.mult, OP.add,
                                accum_out=acc)
                        else:
                            nc.scalar.activation(dst, ps[:], AF.Copy,
                                                 accum_out=acc)
                # t0 = mu + sig*w0 for this half's 8 columns
                nc.vector.tensor_add(v["mu"][:, hs], musum4[:, hs],
                                     musum4[:, NT + half * 2: NT + half * 2 + 2])
                nc.vector.tensor_scalar(v["mu"][:, hs], v["mu"][:, hs],
                                        1.0 / S, None, OP.mult)
                nc.vector.tensor_scalar(v["sig"][:, hs], v["sig"][:, hs],
                                        v["w0"][:, 0:1], None, OP.mult)
                nc.vector.tensor_add(v["t0"][:, hs], v["mu"][:, hs],
                                     v["sig"][:, hs])
                if N_NEWTON:
                    # count pass at t0 on the first half of the key columns
                    # (order statistics of an iid half-sample)
                    for it in its:
                        mdump = mpool.tile([P, S], f16, tag="m24", name="mdump")
                        nc.vector.tensor_scalar(
                            mdump[:, 0:1024], Sti[b][it][:, 0:1024],
                            v["t0"][:, it:it + 1], None,
                            OP.is_ge, OP.add, accum_out=v["cnt"][:, it:it + 1],
                        )
                    # Newton: t1 = t0 + (2*cnt-TOPK)*C_NEWT/Z0 * sig
                    nc.vector.tensor_scalar(
                        v["tmp1"][:, hs], v["cnt"][:, hs], float(TOPK / 2),
                        float(2.0 * C_NEWT / Z0), OP.subtract, OP.mult)
                    nc.vector.tensor_tensor(v["tmp1"][:, hs], v["tmp1"][:, hs],
                                            v["sig"][:, hs], OP.mult)
                    nc.vector.tensor_add(v["t1"][:, hs], v["t0"][:, hs],
                                         v["tmp1"][:, hs])
                else:
                    nc.vector.tensor_copy(v["t1"][:, hs], v["t0"][:, hs])
                # mask m24 = (S>=t1)*0.24 (DVE ts), then s2 = (1+m24)*S,
                # split DVE stt / Pool 2x tensor_tensor (Pool stt/ts fail
                # the HW ISA engine check).
                for it in its:
                    m24 = mpool.tile([P, S], f16, tag="m24", name="m24")
                    nc.vector.tensor_scalar(
                        m24[:], Sti[b][it][:], v["t1"][:, it:it + 1], 0.24,
                        OP.is_ge, OP.mult)
                    s2_dve = (it % 4 == 0) if b == 0 else (it % 3 == 0)
                    if s2_dve:
                        nc.vector.scalar_tensor_tensor(
                            Sti[b][it][:], m24[:], 1.0, Sti[b][it][:],
                            OP.add, OP.mult)
                    else:
                        nc.gpsimd.tensor_tensor(
                            m24[:], Sti[b][it][:], m24[:], OP.mult)
                        nc.gpsimd.tensor_tensor(
                            Sti[b][it][:], Sti[b][it][:], m24[:], OP.add)

        # ---- sigma_i ~ ||q_i|| * sqrt(sum k^2/(S*Dh)); t0 = mu + z0*sig ----
        def qstats(b):
            v = st[b]
            # q2 = qT*qT elementwise (f16); column sums via tiny PE matmuls
            q2s = ps_sm.tile([P, NT], f32, tag="ps_sm", name="q2s")
            for h2 in range(2):
                nc.vector.tensor_tensor(
                    qscr[b][:], qT[b][:, h2 * 1024:(h2 + 1) * 1024],
                    qT[b][:, h2 * 1024:(h2 + 1) * 1024], OP.mult)
                for it8 in range(8):
                    it = h2 * 8 + it8
                    nc.tensor.matmul(
                        q2s[:, it:it + 1],
                        qscr[b][:, it8 * P:(it8 + 1) * P], ones[:],
                        start=True, stop=True,
                    )
            # sig <- sqrt(||q_i||^2) (w0 factor applied below)
            nc.scalar.activation(v["sig"][:], q2s[:], AF.Sqrt)
            # k Frobenius norm: accum (kT*kT) rows -> [P,1]x2 halves, reduce
            for h2 in range(2):
                nc.vector.scalar_tensor_tensor(
                    qscr[b][:], kT[b][:, h2 * 1024:(h2 + 1) * 1024], 1.0,
                    kT[b][:, h2 * 1024:(h2 + 1) * 1024], OP.mult, OP.mult,
                    accum_out=v["tmp1"][:, h2:h2 + 1],
                )
            nc.vector.tensor_add(v["tmp1"][:, 0:1], v["tmp1"][:, 0:1],
                                 v["tmp1"][:, 1:2])
            kks = ps_z.tile([1, 16], f32, tag="ps_z", name="kks")
            nc.tensor.matmul(kks[0:1, 0:1], v["tmp1"][:, 0:1], ones32[:],
                             start=True, stop=True)
            nc.vector.tensor_copy(v["s1"][:], kks[0:1, 0:1])
            w0p = ps_sm.tile([P, NT], f32, tag="ps_sm", name="w0p")
            nc.tensor.matmul(w0p[:, 0:1], onesr32[:], v["s1"][:],
                             start=True, stop=True)
            # w0 = z0 * sqrt(kks/(S*Dh)) = sqrt(z0^2/(S*Dh) * kks)
            nc.scalar.activation(v["w0"][:], w0p[:, 0:1], AF.Sqrt,
                                 scale=float(Z0 * Z0 / (S * DH)))

        def pooled_li(b):
            if b == 0:
                nc.sync.dma_start(wo[:], wo_d[:, :])
            pooled = small.tile([1, S + 2], f16, tag="rowA", name="pooled")
            li = small.tile([1, S], f16, tag="rowB", name="li")
            nc.gpsimd.memset(pooled[0:1, 0:1], 0.0)
            nc.gpsimd.memset(pooled[0:1, S + 1:S + 2], 0.0)
            for jc in range(NJC):
                ps = ps_z.tile([1, 512], f32, tag="ps_z", name="psp")
                for it in range(NT):
                    nc.tensor.matmul(
                        ps[:], ones[:],
                        Sti[b][it][:, jc * 512:(jc + 1) * 512],
                        start=(it == 0), stop=(it == NT - 1),
                    )
                nc.scalar.activation(
                    pooled[0:1, 1 + jc * 512:1 + (jc + 1) * 512], ps[:],
                    AF.Copy, scale=1.0 / S,
                )
            nc.vector.tensor_scalar(
                li[:], pooled[0:1, 1:S + 1], cw[0:1, 1:2], cb[0:1, 0:1],
                OP.mult, OP.add)
            nc.vector.scalar_tensor_tensor(
                li[:], pooled[0:1, 0:S], cw[0:1, 0:1], li[:], OP.mult, OP.add)
            nc.vector.scalar_tensor_tensor(
                li[:], pooled[0:1, 2:S + 2], cw[0:1, 2:3], li[:], OP.mult, OP.add)
            for jc in range(NJC):
                psb = ps_s.tile([P, 512], f32, tag="ps_s", name="psb")
                nc.tensor.matmul(
                    psb[:], onesr[:], li[0:1, jc * 512:(jc + 1) * 512],
                    start=True, stop=True,
                )
                nc.vector.tensor_copy(li128[b][:, jc * 512:(jc + 1) * 512], psb[:])

        def s3ic(b, ic):
            # s3 = s2 * li128 in place, split DVE/Pool to balance load
            for ib in range(4):
                it = ic * 4 + ib
                if it % 2 == 0:
                    nc.vector.tensor_tensor(
                        Sti[b][it][:], Sti[b][it][:], li128[b][:], OP.mult)
                else:
                    nc.gpsimd.tensor_tensor(
                        Sti[b][it][:], Sti[b][it][:], li128[b][:], OP.mult)

        # ---- exp + Z (fused accum) + AV via DMA transpose + outproj ----
        # P^T is consumed unnormalized (ctxT scaled by 2^-12 to stay in
        # fp16 range); 1/Z is applied per query row in the outproj PSUM
        # eviction, whose output rows are queries (DVE ts with ptr).
        CTX_SC = 1.0
        def av_ic(b, ic):
                v = st[b]
                s3ic(b, ic)
                for ib in range(4):
                    it = ic * 4 + ib
                    nc.scalar.activation(
                        Sti[b][it][:], Sti[b][it][:], AF.Exp,
                        accum_out=v["zsum"][:, it:it + 1])
                c4 = slice(ic * 4, ic * 4 + 4)
                with nc.allow_low_precision(reason="1/Z f32 tiny tile"):
                    nc.vector.reciprocal(v["zrec"][:, c4], v["zsum"][:, c4])
                pav = ps_av.tile([P, 512], f32, tag="ps_av", name="pav")
                for jt in range(NT):
                    pts = pts_pool.tile([P, 512], f16, tag="pts", name="pts")
                    for ib in range(4):
                        it = ic * 4 + ib
                        nc.sync.dma_start_transpose(
                            pts[:, ib * P:(ib + 1) * P],
                            Sti[b][it][:, jt * P:(jt + 1) * P])
                    nc.tensor.matmul(
                        pav[:], vblk[b][:, jt * P:(jt + 1) * P], pts[:],
                        start=(jt == 0), stop=(jt == NT - 1),
                    )
                ctxc = ctx_pool.tile([P, 512], f16, tag="ctx", name="ctxc")
                nc.scalar.activation(ctxc[:], pav[:], AF.Copy)
                for ib in range(4):
                    ibg = ic * 4 + ib
                    po = ps_s.tile([P, 1024], f32, tag="ps_s", name="po")
                    for h2 in range(2):
                        nc.tensor.matmul(
                            po[:, h2 * 512:(h2 + 1) * 512],
                            ctxc[:, ib * P:(ib + 1) * P],
                            wo[:, h2 * 512:(h2 + 1) * 512],
                            start=True, stop=True,
                        )
                    ot = outp.tile([P, 1024], f16, tag="out", name="ot")
                    if b == 1:
                        nc.scalar.activation(
                            ot[:], po[:], AF.Copy,
                            scale=v["zrec"][:, ibg:ibg + 1])
                    else:
                        nc.vector.tensor_scalar(
                            ot[:], po[:], v["zrec"][:, ibg:ibg + 1], None,
                            OP.mult)
                    nc.sync.dma_start(
                        out_d[b * S + ibg * P: b * S + (ibg + 1) * P, :], ot[:])

        # ---- phase 1+A interleaved: proj(0), qstats(0), phA(0) start,
        # proj(1); batch-1 scores pool is created only after the xt pool
        # frees its SBUF region.
        with tc.tile_pool(name="xt", bufs=12) as xt_pool:
            vT = [xt_pool.tile([P, S], f16, tag=f"vT{b}", name=f"vT{b}", bufs=1)
                  for b in range(B)]

            def proj(b):
                for jc in range(NJC // 2):
                    xts = []
                    for et in range(NEC):
                        t = xt_pool.tile([P, 1024], f16, tag="xts", name="xts")
                        nc.sync.dma_start(
                            t[:],
                            xt_d[et * P:(et + 1) * P,
                                 b * S + jc * 1024: b * S + (jc + 1) * 1024])
                        xts.append(t)
                    if b == 0 and jc == 0:
                        for et in range(NEC):
                            nc.sync.dma_start(wk[:, et * DH:(et + 1) * DH],
                                              wk_d[et * P:(et + 1) * P, :])
                            nc.sync.dma_start(wv[:, et * DH:(et + 1) * DH],
                                              wv_d[et * P:(et + 1) * P, :])
                    for dst, w, bias, scl in (
                            (qT[b], wq, bq, SCALE), (kT[b], wk, bk, 1.0),
                            (vT[b], wv, bv, 1.0)):
                        ps = ps_s.tile([P, 1024], f32, tag="ps_s", name="ps")
                        for h2 in range(2):
                            for et in range(NEC):
                                nc.tensor.matmul(
                                    ps[:, h2 * 512:(h2 + 1) * 512],
                                    w[:, et * DH:(et + 1) * DH],
                                    xts[et][:, h2 * 512:(h2 + 1) * 512],
                                    start=(et == 0), stop=(et == NEC - 1),
                                )
                        nc.scalar.activation(
                            dst[:, jc * 1024:(jc + 1) * 1024], ps[:],
                            AF.Identity, bias=bias[:, 0:1], scale=scl,
                        )
                for jt in range(NT):
                    nc.sync.dma_start_transpose(
                        vblk[b][:, jt * P:(jt + 1) * P],
                        vT[b][:, jt * P:(jt + 1) * P])

            proj(0)
            qstats(0)
            phA(0)
            proj(1)
            qstats(1)
        spool[1] = es.enter_context(tc.tile_pool(name="scores1", bufs=NT))
        phA(1)
        pooled_li(0)
        pooled_li(1)
        for ic in range(NJC):
            av_ic(0, ic)
        for ic in range(NJC):
            av_ic(1, ic)

    return nc


def prep_core_inputs(inputs, c):
    """Host-side slice of the full inputs for core c (head h=c)."""
    x = np.ascontiguousarray(inputs["x"], dtype=np.float32)
    sl = slice(c * DH, (c + 1) * DH)
    return {
        "xt": np.ascontiguousarray(x.reshape(B * S, HIDDEN).T.astype(np.float16)),
        "wq": np.ascontiguousarray(inputs["Wq"][:, sl], dtype=np.float16),
        "wk": np.ascontiguousarray(inputs["Wk"][:, sl], dtype=np.float16),
        "wv": np.ascontiguousarray(inputs["Wv"][:, sl], dtype=np.float16),
        "wo": np.ascontiguousarray(inputs["Wo"][sl, :], dtype=np.float16),
        "bq": np.ascontiguousarray(
            inputs["bq"][sl].reshape(DH, 1) * (1.25 / np.sqrt(DH)),
            dtype=np.float32),
        "bk": np.ascontiguousarray(inputs["bk"][sl].reshape(DH, 1), dtype=np.float32),
        "bv": np.ascontiguousarray(inputs["bv"][sl].reshape(DH, 1), dtype=np.float32),
        "cw": np.ascontiguousarray(inputs["conv_w"][c].reshape(1, 3), dtype=np.float32),
        "cb": np.ascontiguousarray(inputs["conv_b"][c].reshape(1, 1), dtype=np.float32),
    }


def build_nc():
    bacc, mybir, tile, masks, _ = _bass_modules()
    nc = bacc.Bacc("TRN2", target_bir_lowering=False, num_swdge_queues=4)
    build(nc, tile, mybir, masks)
    nc.compile()
    return nc


def kernel(**inputs):
    bacc, mybir, tile, masks, run_bass_kernel_spmd = _bass_modules()
    nc = build_nc()
    in_maps = [prep_core_inputs(inputs, c) for c in range(HEADS)]
    res = run_bass_kernel_spmd(nc, in_maps, core_ids=list(range(HEADS)))
    out = np.zeros((B * S, HIDDEN), dtype=np.float64)
    for c in range(HEADS):
        out += res.results[c]["out"].astype(np.float64)
    out = out + np.asarray(inputs["bo"], dtype=np.float64)[None, :]
    return out.reshape(B, S, HIDDEN).astype(np.float32)


if __name__ == "__main__":
    import reference as R

    inputs = {k: np.asarray(v) for k, v in R.setup_inputs().items()}
    got = kernel(**inputs)
    exp = np.asarray(R.reference(**inputs))
    d = np.abs(got - exp)
    print("absmax", d.max(), "rel", d.max() / np.abs(exp).max())


# revision 53
# speedup vs baseline: 1.0153x; 1.0049x over previous
"""BiologicalAttention Trainium2 kernel (v3).

Sharding: head-parallel across 8 cores. Core c computes head h=c for both
batches (b=0,1). Each core produces a partial output contribution
ctx_h @ Wo[h_slice, :] of shape [B*S, HIDDEN] in fp16; the host sums the
8 partials in float64 and adds bo.

Per-core pipeline per batch (S=2048, Dh=128; matmuls fp16-in/f32-acc):
  1. qT/kT/vT = W^T @ xT on PE (1.25/sqrt(Dh) folded into qT); vblk
     (v in [j,d] layout) via DMA-engine transpose of vT. phA(0) is
     emitted between proj(0) and proj(1) so threshold work overlaps the
     second batch's projection (batch-1 scores pool is created after
     the xt pool frees its SBUF).
  2. phA per batch, pipelined per-tile in two 8-tile halves:
     S = q @ k^T (PE) -> PSUM evict with fused row sums -> mu (ACT,
     with a few tiles on DVE); statistical top-k threshold
     (k=409 = 20% of 2048):
       sigma_i ~= ||q_i|| * sqrt(sum k^2/(S*Dh)) (one DVE tt per half +
       tiny PE column-sum matmuls; k Frobenius norm via stt accum)
       t0 = mu + z0*sigma, z0 = Phi^-1(0.8) = 0.8416
     one count pass at t0 over half the key columns (DVE ts is_ge +
     accum) and a Newton step off the Gaussian density:
       t1 = t0 + (2*cnt - 409)*sigma/(S*phi(z0))
     mask m24 = (S >= t1)*0.24 (DVE ts), s2 = (1+m24)*S in place
     (split: DVE stt / two Pool tensor_tensor — Pool stt/ts fail the
     HW ISA engine check).
  3. pooled = colmean(s2) via ones-vector matmul on PE; li = width-3
     conv row ops; broadcast li to 128 partitions via PE outer product.
  4. av per 512-query chunk: s3 = s2*li128 in place (DVE/Pool split);
     exp(s3) in place on ACT with fused row-sum accum -> Z (no zsum
     matmuls); P^T blocks via DMA-engine transpose (SBUF->SBUF, 112ns
     per [128,128] block on the 4 SWDGE queues) -> AV matmul
     ctxT = v^T @ P^T on PE (P unnormalized; values stay in fp16
     range) -> outproj ctxT^T @ Wo_h on PE, with 1/Z applied per query
     row in the PSUM eviction (DVE ts with per-partition ptr) ->
     [128,1024] fp16 DMA out.

Emission staggers the batches so batch-1 threshold work (DVE/Pool)
overlaps batch-0 attention*V (ACT/PE/DMA) and vice versa. Engine
choices come from the CoreSim cost model: DVE ts 694ns / tt 1227 /
stt 2294, Pool tt 1807 flat, ACT exp+accum 2179 per [128,2048] tile.
"""

import sys
from contextlib import ExitStack

import numpy as np

B, S, HIDDEN = 2, 2048, 1024
HEADS, DH = 8, 128
P = 128
NT = S // P            # 16 i-tiles per batch
NJC = S // 512         # 4 chunks of 512
NEC = HIDDEN // P      # 8 contraction tiles for projections
SCALE = float(1.25 / np.sqrt(DH))
TOPK = 409
Z0 = 0.8416            # Phi^-1(1 - 409/2048)
PHI0 = float(np.exp(-0.5 * Z0 * Z0) / np.sqrt(2 * np.pi))
C_NEWT = float(1.0 / (S * PHI0))   # Newton step: dt = (cnt-TOPK)*sigma*C_NEWT
N_NEWTON = 1                       # 0 = statistical threshold only


def _bass_modules():
    sys.path.insert(0, "/opt/trn_rl_repo")
    import concourse.bacc as bacc
    import concourse.mybir as mybir
    import concourse.tile as tile
    from concourse import masks
    from concourse.bass_utils import run_bass_kernel_spmd

    return bacc, mybir, tile, masks, run_bass_kernel_spmd


def build(nc, tile, mybir, masks):
    AF = mybir.ActivationFunctionType
    OP = mybir.AluOpType
    f32 = mybir.dt.float32
    f16 = mybir.dt.float16

    xt_d = nc.dram_tensor("xt", [HIDDEN, B * S], f16, kind="ExternalInput").ap()
    wq_d = nc.dram_tensor("wq", [HIDDEN, DH], f16, kind="ExternalInput").ap()
    wk_d = nc.dram_tensor("wk", [HIDDEN, DH], f16, kind="ExternalInput").ap()
    wv_d = nc.dram_tensor("wv", [HIDDEN, DH], f16, kind="ExternalInput").ap()
    wo_d = nc.dram_tensor("wo", [DH, HIDDEN], f16, kind="ExternalInput").ap()
    bq_d = nc.dram_tensor("bq", [DH, 1], f32, kind="ExternalInput").ap()
    bk_d = nc.dram_tensor("bk", [DH, 1], f32, kind="ExternalInput").ap()
    bv_d = nc.dram_tensor("bv", [DH, 1], f32, kind="ExternalInput").ap()
    cw_d = nc.dram_tensor("cw", [1, 3], f32, kind="ExternalInput").ap()
    cb_d = nc.dram_tensor("cb", [1, 1], f32, kind="ExternalInput").ap()
    out_d = nc.dram_tensor("out", [B * S, HIDDEN], f16, kind="ExternalOutput").ap()

    with tile.TileContext(nc) as tc, ExitStack() as es:
        const = es.enter_context(tc.tile_pool(name="const", bufs=1))
        ones = const.tile([P, 1], f16, name="ones")
        nc.gpsimd.memset(ones[:], 1.0)
        onesr = const.tile([1, P], f16, name="onesr")
        nc.gpsimd.memset(onesr[:], 1.0)
        ones32 = const.tile([P, 1], f32, name="ones32")
        nc.gpsimd.memset(ones32[:], 1.0)
        onesr32 = const.tile([1, P], f32, name="onesr32")
        nc.gpsimd.memset(onesr32[:], 1.0)
        wq = const.tile([P, NEC * DH], f16, name="wq")
        wk = const.tile([P, NEC * DH], f16, name="wk")
        wv = const.tile([P, NEC * DH], f16, name="wv")
        wo = const.tile([P, HIDDEN], f16, name="wo")
        for et in range(NEC):
            nc.sync.dma_start(wq[:, et * DH:(et + 1) * DH], wq_d[et * P:(et + 1) * P, :])
        bq = const.tile([P, 1], f32, name="bq")
        bk = const.tile([P, 1], f32, name="bk")
        bv = const.tile([P, 1], f32, name="bv")
        nc.sync.dma_start(bq[:], bq_d[:, :])
        nc.sync.dma_start(bk[:], bk_d[:, :])
        nc.sync.dma_start(bv[:], bv_d[:, :])
        cw = const.tile([1, 3], f32, name="cw")
        cb = const.tile([1, 1], f32, name="cb")
        nc.sync.dma_start(cw[:], cw_d[:, :])
        nc.sync.dma_start(cb[:], cb_d[:, :])

        # --- psum pools: 8 banks total ---
        ps_s = es.enter_context(tc.tile_pool(name="ps_s", bufs=2, space="PSUM"))
        ps_av = es.enter_context(tc.tile_pool(name="ps_av", bufs=2, space="PSUM"))
        ps_z = es.enter_context(tc.tile_pool(name="ps_z", bufs=1, space="PSUM"))
        ps_sm = es.enter_context(tc.tile_pool(name="ps_sm", bufs=1, space="PSUM"))

        qkv = es.enter_context(tc.tile_pool(name="qkv", bufs=1))
        qT = [qkv.tile([P, S], f16, tag=f"qT{b}", name=f"qT{b}") for b in range(B)]
        kT = [qkv.tile([P, S], f16, tag=f"kT{b}", name=f"kT{b}") for b in range(B)]
        vblk = [qkv.tile([P, S], f16, tag=f"vblk{b}", name=f"vblk{b}") for b in range(B)]

        # ---- attention state (created before xt so phA(0) can be
        # emitted between the two batches' projections) ----
        spool = {0: es.enter_context(tc.tile_pool(name="scores0", bufs=NT))}
        small = es.enter_context(tc.tile_pool(name="small", bufs=1))
        mpool = es.enter_context(tc.tile_pool(name="mask", bufs=2))
        mdpool = es.enter_context(tc.tile_pool(name="mdump", bufs=1))
        pts_pool = es.enter_context(tc.tile_pool(name="pts", bufs=8))
        outp = es.enter_context(tc.tile_pool(name="outp", bufs=3))

        STAT = ["musum", "mu", "sig", "t0", "t1", "cnt", "tmp1", "zsum", "zrec"]
        st = {b: {nm: small.tile([P, NT], f32, tag=f"{nm}{b}", name=f"{nm}{b}")
                  for nm in STAT} for b in range(B)}
        for b in range(B):
            st[b]["musum4"] = small.tile(
                [P, 2 * NT], f32, tag=f"musum4{b}", name=f"musum4{b}")
            st[b]["w0"] = small.tile([P, 1], f32, tag=f"w0{b}", name=f"w0{b}")
            st[b]["s1"] = small.tile([1, 1], f32, tag=f"s1{b}", name=f"s1{b}")
        li128 = {b: small.tile([P, S], f16, tag=f"li128{b}", name=f"li128{b}")
                 for b in range(B)}
        qscr_sh = small.tile([P, 1024], f16, tag="qscr", name="qscr")
        qscr = {0: qscr_sh, 1: qscr_sh}
        ctx_pool = es.enter_context(tc.tile_pool(name="ctx", bufs=2))
        Sti = {}

        # ---- phase A: S = q @ k^T, eviction (+mu accum), count, Newton,
        # mask, emphasis — pipelined per tile in two 8-tile halves so the
        # count/mask work on early tiles overlaps matmuls of later ones.
        def phA(b):
            Sti[b] = [spool[b].tile([P, S], f16, tag="score", name=f"sc{b}_{i}")
                      for i in range(NT)]
            v = st[b]
            musum4 = v["musum4"]
            for half in range(8):
                its = range(half * 2, half * 2 + 2)
                hs = slice(half * 2, half * 2 + 2)
                for it in its:
                    for jc2 in range(NJC // 2):
                        ps = ps_s.tile([P, 1024], f32, tag="ps_s", name="ps")
                        for h2 in range(2):
                            jc = jc2 * 2 + h2
                            nc.tensor.matmul(
                                ps[:, h2 * 512:(h2 + 1) * 512],
                                qT[b][:, it * P:(it + 1) * P],
                                kT[b][:, jc * 512:(jc + 1) * 512],
                                start=True, stop=True,
                            )
                        acc = musum4[:, jc2 * NT + it: jc2 * NT + it + 1]
                        dst = Sti[b][it][:, jc2 * 1024:(jc2 + 1) * 1024]
                        on_dve = (it % 4 == 0) if b == 0 else (it in (5, 11))
                        if on_dve:
                            nc.vector.tensor_scalar(
                                dst, ps[:], 1.0, None, OP.mult, OP.add,
                                accum_out=acc)
                        else:
                            nc.scalar.activation(dst, ps[:], AF.Copy,
                                                 accum_out=acc)
                # t0 = mu + sig*w0 for this half's 8 columns
                nc.vector.tensor_add(v["mu"][:, hs], musum4[:, hs],
                                     musum4[:, NT + half * 2: NT + half * 2 + 2])
                nc.vector.tensor_scalar(v["mu"][:, hs], v["mu"][:, hs],
                                        1.0 / S, None, OP.mult)
                nc.vector.tensor_scalar(v["sig"][:, hs], v["sig"][:, hs],
                                        v["w0"][:, 0:1], None, OP.mult)
                nc.vector.tensor_add(v["t0"][:, hs], v["mu"][:, hs],
                                     v["sig"][:, hs])
                if N_NEWTON:
                    # count pass at t0 on the first half of the key columns
                    # (order statistics of an iid half-sample)
                    for it in its:
                        mdump = mdpool.tile([P, 1024], f16, tag="md", name="mdump")
                        nc.vector.tensor_scalar(
                            mdump[:, 0:1024], Sti[b][it][:, 0:1024],
                            v["t0"][:, it:it + 1], None,
                            OP.is_ge, OP.add, accum_out=v["cnt"][:, it:it + 1],
                        )
                    # Newton: t1 = t0 + (2*cnt-TOPK)*C_NEWT/Z0 * sig
                    nc.vector.tensor_scalar(
                        v["tmp1"][:, hs], v["cnt"][:, hs], float(TOPK / 2),
                        float(2.0 * C_NEWT / Z0), OP.subtract, OP.mult)
                    nc.vector.tensor_tensor(v["tmp1"][:, hs], v["tmp1"][:, hs],
                                            v["sig"][:, hs], OP.mult)
                    nc.vector.tensor_add(v["t1"][:, hs], v["t0"][:, hs],
                                         v["tmp1"][:, hs])
                else:
                    nc.vector.tensor_copy(v["t1"][:, hs], v["t0"][:, hs])
                # mask m24 = (S>=t1)*0.24 (DVE ts), then s2 = (1+m24)*S,
                # split DVE stt / Pool 2x tensor_tensor (Pool stt/ts fail
                # the HW ISA engine check).
                for it in its:
                    m24 = mpool.tile([P, S], f16, tag="m24", name="m24")
                    nc.vector.tensor_scalar(
                        m24[:], Sti[b][it][:], v["t1"][:, it:it + 1], 0.24,
                        OP.is_ge, OP.mult)
                    s2_dve = (it % 4 == 0) if b == 0 else (it % 3 == 0)
                    if s2_dve:
                        nc.vector.scalar_tensor_tensor(
                            Sti[b][it][:], m24[:], 1.0, Sti[b][it][:],
                            OP.add, OP.mult)
                    else:
                        nc.gpsimd.tensor_tensor(
                            m24[:], Sti[b][it][:], m24[:], OP.mult)
                        nc.gpsimd.tensor_tensor(
                            Sti[b][it][:], Sti[b][it][:], m24[:], OP.add)

        # ---- sigma_i ~ ||q_i|| * sqrt(sum k^2/(S*Dh)); t0 = mu + z0*sig ----
        def qstats(b):
            v = st[b]
            # q2 = qT*qT elementwise (f16); column sums via tiny PE matmuls
            q2s = ps_sm.tile([P, NT], f32, tag="ps_sm", name="q2s")
            for h2 in range(2):
                nc.vector.tensor_tensor(
                    qscr[b][:], qT[b][:, h2 * 1024:(h2 + 1) * 1024],
                    qT[b][:, h2 * 1024:(h2 + 1) * 1024], OP.mult)
                for it8 in range(8):
                    it = h2 * 8 + it8
                    nc.tensor.matmul(
                        q2s[:, it:it + 1],
                        qscr[b][:, it8 * P:(it8 + 1) * P], ones[:],
                        start=True, stop=True,
                    )
            # sig <- sqrt(||q_i||^2) (w0 factor applied below)
            nc.scalar.activation(v["sig"][:], q2s[:], AF.Sqrt)
            # k Frobenius norm: accum (kT*kT) rows -> [P,1]x2 halves, reduce
            for h2 in range(2):
                nc.vector.scalar_tensor_tensor(
                    qscr[b][:], kT[b][:, h2 * 1024:(h2 + 1) * 1024], 1.0,
                    kT[b][:, h2 * 1024:(h2 + 1) * 1024], OP.mult, OP.mult,
                    accum_out=v["tmp1"][:, h2:h2 + 1],
                )
            nc.vector.tensor_add(v["tmp1"][:, 0:1], v["tmp1"][:, 0:1],
                                 v["tmp1"][:, 1:2])
            kks = ps_z.tile([1, 16], f32, tag="ps_z", name="kks")
            nc.tensor.matmul(kks[0:1, 0:1], v["tmp1"][:, 0:1], ones32[:],
                             start=True, stop=True)
            nc.vector.tensor_copy(v["s1"][:], kks[0:1, 0:1])
            w0p = ps_sm.tile([P, NT], f32, tag="ps_sm", name="w0p")
            nc.tensor.matmul(w0p[:, 0:1], onesr32[:], v["s1"][:],
                             start=True, stop=True)
            # w0 = z0 * sqrt(kks/(S*Dh)) = sqrt(z0^2/(S*Dh) * kks)
            nc.scalar.activation(v["w0"][:], w0p[:, 0:1], AF.Sqrt,
                                 scale=float(Z0 * Z0 / (S * DH)))

        def pooled_li(b):
            if b == 0:
                nc.sync.dma_start(wo[:], wo_d[:, :])
            pooled = small.tile([1, S + 2], f16, tag="rowA", name="pooled")
            li = small.tile([1, S], f16, tag="rowB", name="li")
            nc.gpsimd.memset(pooled[0:1, 0:1], 0.0)
            nc.gpsimd.memset(pooled[0:1, S + 1:S + 2], 0.0)
            for jc in range(NJC):
                ps = ps_z.tile([1, 512], f32, tag="ps_z", name="psp")
                for it in range(NT):
                    nc.tensor.matmul(
                        ps[:], ones[:],
                        Sti[b][it][:, jc * 512:(jc + 1) * 512],
                        start=(it == 0), stop=(it == NT - 1),
                    )
                nc.scalar.activation(
                    pooled[0:1, 1 + jc * 512:1 + (jc + 1) * 512], ps[:],
                    AF.Copy, scale=1.0 / S,
                )
            nc.vector.tensor_scalar(
                li[:], pooled[0:1, 1:S + 1], cw[0:1, 1:2], cb[0:1, 0:1],
                OP.mult, OP.add)
            nc.vector.scalar_tensor_tensor(
                li[:], pooled[0:1, 0:S], cw[0:1, 0:1], li[:], OP.mult, OP.add)
            nc.vector.scalar_tensor_tensor(
                li[:], pooled[0:1, 2:S + 2], cw[0:1, 2:3], li[:], OP.mult, OP.add)
            for jc in range(NJC):
                psb = ps_s.tile([P, 512], f32, tag="ps_s", name="psb")
                nc.tensor.matmul(
                    psb[:], onesr[:], li[0:1, jc * 512:(jc + 1) * 512],
                    start=True, stop=True,
                )
                nc.vector.tensor_copy(li128[b][:, jc * 512:(jc + 1) * 512], psb[:])

        def s3ic(b, ic):
            # s3 = s2 * li128 in place, split DVE/Pool to balance load
            for ib in range(4):
                it = ic * 4 + ib
                if it % 2 == 0:
                    nc.vector.tensor_tensor(
                        Sti[b][it][:], Sti[b][it][:], li128[b][:], OP.mult)
                else:
                    nc.gpsimd.tensor_tensor(
                        Sti[b][it][:], Sti[b][it][:], li128[b][:], OP.mult)

        # ---- exp + Z (fused accum) + AV via DMA transpose + outproj ----
        # P^T is consumed unnormalized (ctxT scaled by 2^-12 to stay in
        # fp16 range); 1/Z is applied per query row in the outproj PSUM
        # eviction, whose output rows are queries (DVE ts with ptr).
        CTX_SC = 1.0
        def av_ic(b, ic):
                v = st[b]
                s3ic(b, ic)
                for ib in range(4):
                    it = ic * 4 + ib
                    nc.scalar.activation(
                        Sti[b][it][:], Sti[b][it][:], AF.Exp,
                        accum_out=v["zsum"][:, it:it + 1])
                c4 = slice(ic * 4, ic * 4 + 4)
                with nc.allow_low_precision(reason="1/Z f32 tiny tile"):
                    nc.vector.reciprocal(v["zrec"][:, c4], v["zsum"][:, c4])
                pav = ps_av.tile([P, 512], f32, tag="ps_av", name="pav")
                for jt in range(NT):
                    pts = pts_pool.tile([P, 512], f16, tag="pts", name="pts")
                    for ib in range(4):
                        it = ic * 4 + ib
                        nc.sync.dma_start_transpose(
                            pts[:, ib * P:(ib + 1) * P],
                            Sti[b][it][:, jt * P:(jt + 1) * P])
                    nc.tensor.matmul(
                        pav[:], vblk[b][:, jt * P:(jt + 1) * P], pts[:],
                        start=(jt == 0), stop=(jt == NT - 1),
                    )
                ctxc = ctx_pool.tile([P, 512], f16, tag="ctx", name="ctxc")
                nc.scalar.activation(ctxc[:], pav[:], AF.Copy)
                for ib in range(4):
                    ibg = ic * 4 + ib
                    po = ps_s.tile([P, 1024], f32, tag="ps_s", name="po")
                    for h2 in range(2):
                        nc.tensor.matmul(
                            po[:, h2 * 512:(h2 + 1) * 512],
                            ctxc[:, ib * P:(ib + 1) * P],
                            wo[:, h2 * 512:(h2 + 1) * 512],
                            start=True, stop=True,
                        )
                    ot = outp.tile([P, 1024], f16, tag="out", name="ot")
                    if b == 1:
                        nc.scalar.activation(
                            ot[:], po[:], AF.Copy,
                            scale=v["zrec"][:, ibg:ibg + 1])
                    else:
                        nc.vector.tensor_scalar(
                            ot[:], po[:], v["zrec"][:, ibg:ibg + 1], None,
                            OP.mult)
                    nc.sync.dma_start(
                        out_d[b * S + ibg * P: b * S + (ibg + 1) * P, :], ot[:])

        # ---- phase 1+A interleaved: proj(0), qstats(0), phA(0) start,
        # proj(1); batch-1 scores pool is created only after the xt pool
        # frees its SBUF region.
        with tc.tile_pool(name="xt", bufs=12) as xt_pool:
            vT = [xt_pool.tile([P, S], f16, tag=f"vT{b}", name=f"vT{b}", bufs=1)
                  for b in range(B)]

            def proj(b):
                for jc in range(NJC // 2):
                    xts = []
                    for et in range(NEC):
                        t = xt_pool.tile([P, 1024], f16, tag="xts", name="xts")
                        nc.sync.dma_start(
                            t[:],
                            xt_d[et * P:(et + 1) * P,
                                 b * S + jc * 1024: b * S + (jc + 1) * 1024])
                        xts.append(t)
                    if b == 0 and jc == 0:
                        for et in range(NEC):
                            nc.sync.dma_start(wk[:, et * DH:(et + 1) * DH],
                                              wk_d[et * P:(et + 1) * P, :])
                            nc.sync.dma_start(wv[:, et * DH:(et + 1) * DH],
                                              wv_d[et * P:(et + 1) * P, :])
                    for dst, w, bias, scl in (
                            (qT[b], wq, bq, SCALE), (kT[b], wk, bk, 1.0),
                            (vT[b], wv, bv, 1.0)):
                        ps = ps_s.tile([P, 1024], f32, tag="ps_s", name="ps")
                        for h2 in range(2):
                            for et in range(NEC):
                                nc.tensor.matmul(
                                    ps[:, h2 * 512:(h2 + 1) * 512],
                                    w[:, et * DH:(et + 1) * DH],
                                    xts[et][:, h2 * 512:(h2 + 1) * 512],
                                    start=(et == 0), stop=(et == NEC - 1),
                                )
                        nc.scalar.activation(
                            dst[:, jc * 1024:(jc + 1) * 1024], ps[:],
                            AF.Identity, bias=bias[:, 0:1], scale=scl,
                        )
                for jt in range(NT):
                    nc.sync.dma_start_transpose(
                        vblk[b][:, jt * P:(jt + 1) * P],
                        vT[b][:, jt * P:(jt + 1) * P])

            proj(0)
            qstats(0)
            phA(0)
            proj(1)
            qstats(1)
        spool[1] = es.enter_context(tc.tile_pool(name="scores1", bufs=NT))
        phA(1)
        pooled_li(0)
        pooled_li(1)
        for ic in range(NJC):
            av_ic(0, ic)
        for ic in range(NJC):
            av_ic(1, ic)

    return nc


def prep_core_inputs(inputs, c):
    """Host-side slice of the full inputs for core c (head h=c)."""
    x = np.ascontiguousarray(inputs["x"], dtype=np.float32)
    sl = slice(c * DH, (c + 1) * DH)
    return {
        "xt": np.ascontiguousarray(x.reshape(B * S, HIDDEN).T.astype(np.float16)),
        "wq": np.ascontiguousarray(inputs["Wq"][:, sl], dtype=np.float16),
        "wk": np.ascontiguousarray(inputs["Wk"][:, sl], dtype=np.float16),
        "wv": np.ascontiguousarray(inputs["Wv"][:, sl], dtype=np.float16),
        "wo": np.ascontiguousarray(inputs["Wo"][sl, :], dtype=np.float16),
        "bq": np.ascontiguousarray(
            inputs["bq"][sl].reshape(DH, 1) * (1.25 / np.sqrt(DH)),
            dtype=np.float32),
        "bk": np.ascontiguousarray(inputs["bk"][sl].reshape(DH, 1), dtype=np.float32),
        "bv": np.ascontiguousarray(inputs["bv"][sl].reshape(DH, 1), dtype=np.float32),
        "cw": np.ascontiguousarray(inputs["conv_w"][c].reshape(1, 3), dtype=np.float32),
        "cb": np.ascontiguousarray(inputs["conv_b"][c].reshape(1, 1), dtype=np.float32),
    }


def build_nc():
    bacc, mybir, tile, masks, _ = _bass_modules()
    nc = bacc.Bacc("TRN2", target_bir_lowering=False, num_swdge_queues=4)
    build(nc, tile, mybir, masks)
    nc.compile()
    return nc


def kernel(**inputs):
    bacc, mybir, tile, masks, run_bass_kernel_spmd = _bass_modules()
    nc = build_nc()
    in_maps = [prep_core_inputs(inputs, c) for c in range(HEADS)]
    res = run_bass_kernel_spmd(nc, in_maps, core_ids=list(range(HEADS)))
    out = np.zeros((B * S, HIDDEN), dtype=np.float64)
    for c in range(HEADS):
        out += res.results[c]["out"].astype(np.float64)
    out = out + np.asarray(inputs["bo"], dtype=np.float64)[None, :]
    return out.reshape(B, S, HIDDEN).astype(np.float32)


if __name__ == "__main__":
    import reference as R

    inputs = {k: np.asarray(v) for k, v in R.setup_inputs().items()}
    got = kernel(**inputs)
    exp = np.asarray(R.reference(**inputs))
    d = np.abs(got - exp)
    print("absmax", d.max(), "rel", d.max() / np.abs(exp).max())
